# revision 34
# baseline (speedup 1.0000x reference)
"""Causal self-attention (B=2, L=4096, D=768, H=12) on 8 TRN2 NeuronCores.

Sharding: core c -> batch b = c//4, head group g = c%4 (heads 3g..3g+2).
No collectives: each core writes its 3 heads' partial output projection for
the FULL sequence; the host sums the 4 head-group cores of each batch
(that is the unshard step) and adds bo.

Per-core structure (QB=256 q-blocks, 128-wide k-tiles, chunks of <=6):
- q/k stored per head as [128, 2L] with the lower/upper partition halves
  duplicated, so consecutive score matmuls (K=64) alternate PE row groups
  (0,0)/(64,0) and run CONCURRENTLY in the array (~2x score throughput).
  Chunk slots are permuted so each concurrent pair writes different PSUM
  banks.
- Two-stream staggered flash attention (A/B half-chunks) keeps the scalar
  engine's exp saturated: it is the critical path (~28M exp elements at
  1 elem/lane/cycle @1.2GHz ~= 190us minimum).
- PSUM: scores A (3 banks) + scores B (3) + shared pyA|pyB rowsum bank (1)
  + projection bank (1) = 8. QKV projection, v projection and the output
  projection are woven into the attention half-chunk slots as PE filler so
  the tensor engine never idles (keeps the HAM clock gate at 8/8).
- attn@v fuses the softmax denominator as a 65th ones-column of v; the
  reciprocal is broadcast across partitions with a K=1 matmul.
Host reassembles [2, 4096, 768] = sum of per-core partials + bo.
"""

import sys

for _p in ("/opt/trn_rl_repo",):
    if _p not in sys.path:
        sys.path.insert(0, _p)

import numpy as np
import ml_dtypes

B, L, D, H = 2, 4096, 768, 12
Dh = D // H          # 64
HPC = 3              # heads per core
NCORES = 8
QB = 256             # q block
KT = 128             # k tile
NQ = L // QB         # 16 q-blocks
KC = D // 128        # 6 contraction chunks for projections
CH = 6               # k-tiles per exp chunk (3 PSUM banks)

# chunk-slot permutations: concurrent score pairs (t, t+1) land in
# different PSUM banks (bank = slot//2); exp spans slots 0..max contiguously
SLOTS = {1: (0,), 2: (0, 2), 3: (0, 2, 1), 4: (0, 2, 1, 3),
         5: (0, 2, 1, 3, 4), 6: (0, 3, 1, 4, 2, 5)}

_CACHE = {}


def _tiles_for_block(b):
    """(kb, w, qo) per k-tile for q-block b (QB=256): the final k-tile is
    half-dead (only q cols 128:256 live), the one before needs the
    triangular mask."""
    out = []
    for kb in range(2 * b + 2):
        if kb == 2 * b + 1:
            out.append((kb, 128, 128))
        else:
            out.append((kb, 256, 0))
    return out


def _chunks_for_block(b):
    t = _tiles_for_block(b)
    return [t[i:i + CH] for i in range(0, len(t), CH)]


def _build():
    import concourse.mybir as mybir
    import concourse.tile as tile
    from concourse import bacc

    bf16 = mybir.dt.bfloat16
    f32 = mybir.dt.float32
    Exp = mybir.ActivationFunctionType.Exp

    nc = bacc.Bacc("TRN2", target_bir_lowering=False, debug=False,
                   num_devices=NCORES)

    # host pre-reshapes to [128, kc, *] so each load is ONE dma trigger
    # (a dma_start costs ~600ns of issue time on its queue's engine)
    xT = nc.dram_tensor('xT', [128, KC, L], bf16, kind='ExternalInput')
    wqkv = nc.dram_tensor('wqkv', [128, KC, 576], bf16, kind='ExternalInput')
    wo01 = nc.dram_tensor('wo01', [128, D], bf16, kind='ExternalInput')
    wo2 = nc.dram_tensor('wo2', [64, D], bf16, kind='ExternalInput')
    msk = nc.dram_tensor('msk', [KT, 256], bf16, kind='ExternalInput')
    out = nc.dram_tensor('out', [L, D], bf16, kind='ExternalOutput')

    with tile.TileContext(nc) as tc:
        with tc.tile_pool(name='const', bufs=1) as cpool, \
             tc.tile_pool(name='work', bufs=3) as wpool:

            # ---------------- load phase ----------------
            # Concurrent dma descriptors SHARE bandwidth, so only what the
            # first lead-in steps need is queued here (q01/k01 weights +
            # x tokens 0:512 = ~1MB -> ready ~3us after the preamble);
            # everything else is queued later, in dependency order.
            wq_sb = cpool.tile([128, KC, 576], bf16)
            nc.sync.dma_start(out=wq_sb[:, :, 0:256], in_=wqkv[:, :, 0:256])
            xt = cpool.tile([128, KC, L], bf16)

            def x_piece(n):
                nc.sync.dma_start(out=xt[:, :, n * 512:(n + 1) * 512],
                                  in_=xT[:, :, n * 512:(n + 1) * 512])

            x_piece(0)
            tri = cpool.tile([KT, 256], bf16)
            wo01_sb = cpool.tile([128, D], bf16)
            wo2_sb = cpool.tile([64, D], bf16)
            ones = cpool.tile([128, 64], bf16)
            nc.vector.memset(ones[:, :], 1.0)

            # per-head q|k, duplicated across partition halves:
            # [0:64]  = q_h (cols 0:L) | k_h (cols L:2L)
            # [64:128] = same (feeds the (64,0) row-group of score pairs)
            qks = [cpool.tile([128, 2 * L], bf16, name=f'qk{h}')
                   for h in range(HPC)]
            v_sb = cpool.tile([128, L // KT, HPC, 65], bf16)
            nc.vector.memset(v_sb[:, :, :, 64:65], 1.0)
            yt01 = cpool.tile([128, L], bf16)
            yt2 = cpool.tile([64, L], bf16)
            yts = [yt01[0:64, :], yt01[64:128, :], yt2[0:64, :]]

            # ---------------- PSUM + proj/outproj steps ----------------
            pp = None    # set below (psum pool)
            pyAB = None  # [65, 512]: A rowsum block at cols 0:256, B at 256:512

            def qk_step(n, ct, tag='pj'):
                """q/k projection for token block n (512 wide), weight
                column chunk ct (0: q_h0|q_h1, 1: k_h0|k_h1, 2: q_h2|k_h2);
                result copied+duplicated into the qk tiles."""
                ps = pp.tile([128, 512], f32, tag=tag, bufs=1, name='pjqk')
                for kc in range(KC):
                    nc.tensor.matmul(ps[:, 0:512],
                                     wq_sb[:, kc, ct * 128:ct * 128 + 128],
                                     xt[:, kc, n * 512:(n + 1) * 512],
                                     start=(kc == 0), stop=(kc == KC - 1))
                st = wpool.tile([128, 512], bf16, tag='st', name='st')
                nc.vector.tensor_copy(st[:, :], ps[:, 0:512])
                qsl = slice(n * 512, (n + 1) * 512)
                ksl = slice(L + n * 512, L + (n + 1) * 512)
                if ct == 0:
                    dsts = [(qks[0], qsl, 0), (qks[1], qsl, 64)]
                elif ct == 1:
                    dsts = [(qks[0], ksl, 0), (qks[1], ksl, 64)]
                else:
                    dsts = [(qks[2], qsl, 0), (qks[2], ksl, 64)]
                for dst, sl, sp in dsts:
                    # same-partition half on DVE, cross-partition dup via
                    # DMA on the scalar queue (sync is busy streaming x;
                    # gpsimd cannot do SBUF->SBUF)
                    nc.vector.tensor_copy(dst[sp:sp + 64, sl], st[sp:sp + 64, :])
                    nc.scalar.dma_start(out=dst[64 - sp:128 - sp, sl],
                                        in_=st[sp:sp + 64, :])

            def v_step(m, tag='pj'):
                ps = pp.tile([128, 512], f32, tag=tag, bufs=1, name='pjv')
                for kc in range(KC):
                    nc.tensor.matmul(ps[:, 0:192],
                                     xt[:, kc, m * 128:(m + 1) * 128],
                                     wq_sb[:, kc, 384:576],
                                     start=(kc == 0), stop=(kc == KC - 1))
                nc.vector.tensor_copy(v_sb[:, m, :, 0:64], ps[:, 0:192])

            def outproj_wave(m, dj, tag='pj'):
                d0, dw = ((0, 512), (512, 256))[dj]
                tok = m * 128
                ps = pp.tile([128, 512], f32, tag=tag, bufs=1, name='pjo')
                nc.tensor.matmul(ps[:, 0:dw], yt01[:, tok:tok + 128],
                                 wo01_sb[:, d0:d0 + dw], start=True, stop=False)
                nc.tensor.matmul(ps[:, 0:dw], yt2[:, tok:tok + 128],
                                 wo2_sb[:, d0:d0 + dw], start=False, stop=True)
                ot = wpool.tile([128, 512], bf16, tag='ot', name='ot')
                nc.vector.tensor_copy(ot[:, 0:dw], ps[:, 0:dw])
                nc.gpsimd.dma_start(out=out[tok:tok + 128, d0:d0 + dw],
                                    in_=ot[:, 0:dw])

            projq = []

            def emit_step(stp, tag='pj'):
                kind = stp[0]
                if kind == 'qk':
                    qk_step(stp[1], stp[2], tag)
                elif kind == 'v':
                    v_step(stp[1], tag)
                else:
                    outproj_wave(stp[1], stp[2])

            def filler():
                k = 2 if len(projq) > 8 else 1
                for _ in range(min(k, len(projq))):
                    emit_step(projq.pop(0))

            def flush_proj(upto_n):
                # projection steps for token blocks <= upto_n must land
                # before the next group reads them; later steps and outproj
                # waves keep draining through filler()
                i = 0
                while i < len(projq):
                    stp = projq[i]
                    n = stp[1] if stp[0] == 'qk' else (
                        stp[1] // 4 if stp[0] == 'v' else 99)
                    if n <= upto_n:
                        emit_step(projq.pop(i))
                    else:
                        i += 1

            # ---------------- attention ----------------
            def normalize(X, ctx):
                off, hX, bX, ytX, done = ctx[X]
                rs = wpool.tile([1, 256], bf16, tag='rs', name='rs')
                nc.vector.tensor_copy(rs[:, :], pyAB[64:65, off:off + 256])
                # broadcast via the stream's own (just-freed) score bank:
                # the shared proj bank would serialize with woven proj steps
                pb = pp.tile([128, 512], f32, tag='s' + X, bufs=1, name='pb')
                nc.tensor.matmul(pb[0:64, 0:256], ones[0:1, 0:64],
                                 rs[0:1, :], start=True, stop=True)
                rcp = wpool.tile([64, 256], f32, tag='rcp', name='rcp')
                nc.vector.reciprocal_approx_fast(out=rcp[:, :],
                                                 in_=pb[0:64, 0:256])
                nc.vector.tensor_mul(ytX[:, bX * QB:(bX + 1) * QB],
                                     pyAB[0:64, off:off + 256],
                                     rcp[:, :])
                if done is not None:
                    done()

            def attn_pair(hA, bA, hB, bB, doneA=None, doneB=None):
                """Two causal streams, staggered half-chunk pipeline.
                Scores are emitted as row-group-alternating pairs so the PE
                computes two K=64 tiles concurrently."""
                chA = _chunks_for_block(bA)
                chB = _chunks_for_block(bB)
                nch = max(len(chA), len(chB))
                chA = chA + [[]] * (nch - len(chA))
                chB = chB + [[]] * (nch - len(chB))
                seq = []
                for j in range(nch):
                    seq.append(('A', chA[j]))
                    seq.append(('B', chB[j]))
                ctx = {'A': (0, hA, bA, yts[hA], doneA),
                       'B': (256, hB, bB, yts[hB], doneB)}
                # start=True clears the WHOLE psum bank (not just the
                # addressed elements) -> only the very first av of the pair
                # may carry it; the other stream's first write still lands
                # fresh because the clear reset its has_written bits
                first = {'pair': True}
                last_idx = {}
                for h, (X, tiles) in enumerate(seq):
                    if tiles:
                        last_idx[X] = h

                def emit_av(X, tiles, pt, is_last):
                    off, hX, bX, ytX, _ = ctx[X]
                    n = len(tiles)
                    smap = SLOTS[n]
                    for t, (kb, w, qo) in enumerate(tiles):
                        c0 = smap[t] * 256 + qo
                        nc.tensor.matmul(
                            pyAB[0:65, off + qo:off + qo + w],
                            v_sb[:, kb, hX, 0:65], pt[:, c0:c0 + w],
                            start=first['pair'] and t == 0,
                            stop=is_last and t == n - 1,
                            skip_group_check=True)
                        if t == 0:
                            first['pair'] = False
                    if is_last:
                        normalize(X, ctx)

                pend = {}
                for h, (X, tiles) in enumerate(seq):
                    off, hX, bX, ytX, _ = ctx[X]
                    if tiles:
                        n = len(tiles)
                        smap = SLOTS[n]
                        s = pp.tile([128, CH * 256], f32, tag='s' + X,
                                    bufs=1, name='s' + X)
                        qk = qks[hX]
                        for t, (kb, w, qo) in enumerate(tiles):
                            hp = (t % 2) * 64
                            c0 = smap[t] * 256 + qo
                            nc.tensor.matmul(
                                s[:, c0:c0 + w],
                                qk[hp:hp + 64, L + kb * KT:L + (kb + 1) * KT],
                                qk[hp:hp + 64, bX * QB + qo:bX * QB + qo + w],
                                start=True, stop=True)
                    if h - 2 in pend:
                        Xp, tp, ptp = pend.pop(h - 2)
                        emit_av(Xp, tp, ptp, last_idx[Xp] == h - 2)
                    if tiles:
                        span = (max(smap[:n]) + 1) * 256
                        pt = wpool.tile([128, CH * 256], bf16, tag='pt',
                                        bufs=6, name='pt' + X)
                        nc.scalar.activation(pt[:, 0:span], s[:, 0:span], Exp)
                        for t, (kb, w, qo) in enumerate(tiles):
                            c0 = smap[t] * 256
                            if kb == 2 * bX:
                                nc.vector.tensor_mul(pt[:, c0:c0 + 256],
                                                     pt[:, c0:c0 + 256],
                                                     tri[:, 0:256])
                            elif kb == 2 * bX + 1:
                                nc.vector.tensor_mul(pt[:, c0 + 128:c0 + 256],
                                                     pt[:, c0 + 128:c0 + 256],
                                                     tri[:, 0:128])
                        pend[h] = (X, tiles, pt)
                    filler()
                for h in sorted(pend):
                    Xp, tp, ptp = pend[h]
                    emit_av(Xp, tp, ptp, last_idx[Xp] == h)

            # ---------- main loop ----------
            with tc.tile_pool(name='psum', bufs=1, space='PSUM') as pp_:
                pp = pp_
                pyAB = pp.tile([65, 512], f32, tag='py', bufs=1, name='pyAB')
                # lead-in: everything attention group 0 needs, rotating
                # through the (still free) attention psum banks
                # minimal lead-in: just q/k of heads 0,1 for token block 0;
                # everything else group 0 needs drains through the fillers
                # so attention (and the exp pipeline) starts ~20us earlier
                emit_step(('qk', 0, 0), tag='sA')
                emit_step(('qk', 0, 1), tag='sB')
                # second DMA wave only once the critical first transfers
                # are queued (descriptors share bandwidth)
                nc.sync.dma_start(out=wq_sb[:, :, 256:576],
                                  in_=wqkv[:, :, 256:576])
                nc.sync.dma_start(out=tri[:, :], in_=msk[:, :])
                x_piece(1)
                projq.extend([('v', 0), ('v', 1), ('v', 2), ('v', 3),
                              ('qk', 0, 2), ('qk', 1, 0), ('qk', 1, 1),
                              ('qk', 1, 2)])

                def push_proj(n):
                    projq.extend([('qk', n, 0), ('qk', n, 1), ('qk', n, 2),
                                  ('v', 4 * n), ('v', 4 * n + 1),
                                  ('v', 4 * n + 2), ('v', 4 * n + 3)])

                projq.extend([('v', m) for m in (4, 5, 6, 7)])
                x_piece(2)
                nc.sync.dma_start(out=wo01_sb[:, :], in_=wo01[:, :])
                nc.sync.dma_start(out=wo2_sb[:, :], in_=wo2[:, :])
                for p in range(8):
                    # two groups of lookahead so group boundaries never
                    # flush a burst (a flush stalls the next group's ACT)
                    if p + 2 <= 7:
                        push_proj(p + 2)
                    if p + 3 <= 7:
                        x_piece(p + 3)

                    def doneA(p=p):
                        # block 2p fully normalized for all 3 heads ->
                        # its two outproj tiles can go now (overlaps the
                        # h2 B-stream tail)
                        for m in (4 * p, 4 * p + 1):
                            for dj in (0, 1):
                                outproj_wave(m, dj)

                    def doneB(p=p):
                        for m in (4 * p + 2, 4 * p + 3):
                            for dj in (0, 1):
                                projq.append(('op', m, dj))

                    if p < 7:
                        attn_pair(0, 2 * p, 1, 2 * p)
                        attn_pair(0, 2 * p + 1, 1, 2 * p + 1)
                        attn_pair(2, 2 * p, 2, 2 * p + 1,
                                  doneA=doneA, doneB=doneB)
                    else:
                        # last group reordered so only block 15's outproj
                        # trails the final attention
                        attn_pair(0, 14, 1, 14)
                        attn_pair(2, 14, 2, 15, doneA=doneA)
                        attn_pair(0, 15, 1, 15)
                        while projq:
                            emit_step(projq.pop(0))
                        # final waves rotate through the now-idle attention
                        # banks so their psum chains run in parallel
                        for i, (m, dj) in enumerate(
                                [(30, 0), (30, 1), (31, 0), (31, 1)]):
                            outproj_wave(m, dj, tag=('pj', 'sA', 'sB', 'py')[i])
                    flush_proj(p + 1)
                while projq:
                    emit_step(projq.pop(0))
    nc.compile()
    return nc


def kernel(x, Wqkv, bqkv, Wo, bo):
    from concourse.bass_utils import run_bass_kernel_spmd

    if 'nc' not in _CACHE:
        _CACHE['nc'] = _build()
    nc = _CACHE['nc']

    bf = ml_dtypes.bfloat16
    x = np.asarray(x, np.float32)
    Wqkv = np.asarray(Wqkv, np.float32)
    bqkv = np.asarray(bqkv, np.float32)
    Wo = np.asarray(Wo, np.float32)
    bo = np.asarray(bo, np.float32)

    # device graph omits the qkv bias adds (always zeros per problem spec)
    assert np.abs(bqkv).max() == 0.0, "nonzero bqkv unsupported by this kernel"

    scale = 1.0 / np.sqrt(Dh)
    Q, K, V = Wqkv[:, 0:D], Wqkv[:, D:2 * D], Wqkv[:, 2 * D:3 * D]

    # triangular mask [128, 256]: col j live for partition r when j >= r
    msk = np.ascontiguousarray(
        np.arange(256)[None, :] >= np.arange(KT)[:, None]).astype(bf)

    in_maps = []
    for c in range(NCORES):
        b, g = divmod(c, 4)
        hs = [3 * g, 3 * g + 1, 3 * g + 2]
        cols = lambda W, h: W[:, h * Dh:(h + 1) * Dh]
        wqkv_np = np.concatenate(
            [cols(Q, hs[0]) * scale, cols(Q, hs[1]) * scale,
             cols(K, hs[0]), cols(K, hs[1]),
             cols(Q, hs[2]) * scale, cols(K, hs[2]),
             cols(V, hs[0]), cols(V, hs[1]), cols(V, hs[2])],
            axis=1).astype(bf)
        wo01_np = Wo[3 * g * Dh:(3 * g + 2) * Dh, :].astype(bf)
        wo2_np = Wo[(3 * g + 2) * Dh:(3 * g + 3) * Dh, :].astype(bf)
        # [D, *] -> [128, kc, *] so each device-side load is one DMA
        xT_np = np.ascontiguousarray(
            x[b].T.reshape(KC, 128, L).transpose(1, 0, 2)).astype(bf)
        wq_np = np.ascontiguousarray(
            wqkv_np.reshape(KC, 128, 576).transpose(1, 0, 2))
        in_maps.append({
            'xT': xT_np, 'wqkv': wq_np,
            'wo01': np.ascontiguousarray(wo01_np),
            'wo2': np.ascontiguousarray(wo2_np),
            'msk': msk,
        })

    res = run_bass_kernel_spmd(nc, in_maps, core_ids=list(range(NCORES)))

    # each core returns the partial outproj sum for its 3 heads over the
    # full sequence; sum the 4 head-group cores of each batch
    out = np.empty((B, L, D), np.float32)
    for b in range(B):
        acc = res.results[4 * b]['out'].astype(np.float32)
        for g in range(1, 4):
            acc += res.results[4 * b + g]['out'].astype(np.float32)
        out[b] = acc
    out += bo[None, None, :]
    return out
